# revision 47
# baseline (speedup 1.0000x reference)
"""Bidirectional Mamba block on 8 Trainium2 NeuronCores.

Sharding: core c in 0..7 handles (batch b = c % 4, direction d = c // 4).
The two directions of one batch are independent branches until the final
out_proj-sum + residual + RMSNorm, which a second tiny SPMD kernel does
(8 cores = 4 batches x 2 sequence halves).

Stage A (per core), per 1024-column chunk: LayerNorm (PE ones-matmul stats,
f16 rows, sqrt+reciprocal rsqrt) -> in_proj -> causal dwconv as 4 PE
diag(w_k)-matmuls accumulated in PSUM + SiLU-from-PSUM -> x_proj ->
dt_proj+softplus (exp batched ahead of ln to limit activation-table
reloads) -> selective scan: the two e-tiles of each block are fused into
one 2048-wide recurrence (single da exp on ScalarE, single
tensor_tensor_scan on DVE; the segment boundary column of da is zeroed and
the second tile's carry is injected through the matching db column, which
keeps the recurrence exact). Valid only when A is constant across channels
(standard Mamba init) - verified on the host per call, with an unmerged
fallback build. db=du*B mostly on GPSIMD (1/6 on DVE to balance), gg=h*C
on DVE, y accumulation over n via identity matmuls into PSUM seeded by a
diag(D) matmul -> silu(z) gate -> out_proj partial (f16).

The two chunks are software-pipelined at emission level: each chunk's head
(LN/in_proj/conv/x_proj/dt) is spliced into the previous chunk's scan loop
at iteration indices chosen to respect every tile-buffer WAR hazard
(xpad/xc generation cycling, per-i dl/du single buffering, PSUM bank
rotation), and the previous chunk's tail (gate + out_proj) is spliced into
the next scan. DRAM bounce buffers (B/C rows, z spill) ping-pong per chunk
parity so per-tensor DRAM dependency tracking cannot serialize chunks.
Host only does slicing / transposes / flips (layout, no math).
"""

import sys
import numpy as np

sys.path.insert(0, "/opt/trn_rl_repo")

B, L, D, E, N, KC, R = 4, 2048, 512, 1024, 16, 4, 32
EPS = 1e-5
ET = E // 128       # 8 e-tiles
DT = D // 128       # 4 d-tiles
TL = 1024           # L chunk size
BLK = 2             # scan i-block size (ypss PSUM tiles alive at once)
DTG = 2             # dt_proj exp/ln batching group (i's per table-set switch)
NL = L // TL        # chunks
NSUB = TL // 512    # 512-wide matmul subchunks per chunk

_cache = {}


def _build_stage_a(reps=1, merged=True):
    import concourse.tile as tile
    from concourse import bacc, mybir
    from concourse.alu_op_type import AluOpType as op
    from contextlib import ExitStack

    dt = mybir.dt
    f32, f16 = dt.float32, dt.float16
    AF = mybir.ActivationFunctionType

    nc = bacc.Bacc("TRN2", target_bir_lowering=False, debug=False, num_devices=8)

    # ---- DRAM I/O (per-core values supplied via in_maps) ----
    hsT = nc.dram_tensor("hsT", [D, L], f16, kind="ExternalInput").ap()
    w_inT = nc.dram_tensor("w_inT", [D, 2 * E], f16, kind="ExternalInput").ap()
    out_wT = nc.dram_tensor("out_wT", [E, D], f16, kind="ExternalInput").ap()
    xp_wT = nc.dram_tensor("xp_wT", [E, R + 2 * N], f16, kind="ExternalInput").ap()
    dtp_wT = nc.dram_tensor("dtp_wT", [R, E], f16, kind="ExternalInput").ap()
    # packed per-partition columns: [conv_w(4) per tile | conv_b | dt_b | D | norm cols]
    convw = nc.dram_tensor("convw", [128, ET * KC], f32, kind="ExternalInput").ap()
    convb = nc.dram_tensor("convb", [128, ET], f32, kind="ExternalInput").ap()
    dtb = nc.dram_tensor("dtb", [128, ET], f32, kind="ExternalInput").ap()
    dvec = nc.dram_tensor("dvec", [128, ET], f32, kind="ExternalInput").ap()
    avals = nc.dram_tensor("avals", [128, ET * N], f32, kind="ExternalInput").ap()
    nw = nc.dram_tensor("nw", [128, DT], f32, kind="ExternalInput").ap()
    nb = nc.dram_tensor("nb", [128, DT], f32, kind="ExternalInput").ap()
    y_part = nc.dram_tensor("y_part", [D, L], f16, kind="ExternalOutput").ap()
    # ping-pong bounce buffers (parity per chunk) so chunk c+1's writes don't
    # serialize against chunk c's reads via per-tensor DRAM dep tracking
    bcd = [nc.dram_tensor(f"bcd{p}", [2 * N, TL], f16).ap() for p in range(2)]
    zdram = [nc.dram_tensor(f"zdram{p}", [E, TL], f16).ap() for p in range(2)]

    with tile.TileContext(nc) as tc:
        with ExitStack() as ctx:
            P = 128

            def pool(name, bufs):
                return ctx.enter_context(tc.tile_pool(name=name, bufs=bufs))

            pers = pool("pers", 1)
            ps_pool = ctx.enter_context(tc.tile_pool(name="ps", bufs=3, space="PSUM"))
            ps_aux = ctx.enter_context(tc.tile_pool(name="psaux", bufs=1, space="PSUM"))
            ps_y = ctx.enter_context(tc.tile_pool(name="psy", bufs=2, space="PSUM"))

            # ---- persistent weight tiles ----
            w_in = [pers.tile([P, 2 * E], f16, tag=f"win{k}", name=f"win{k}") for k in range(DT)]
            for k in range(DT):
                nc.sync.dma_start(w_in[k][:], w_inT[128 * k:128 * (k + 1), :])
            out_w = [pers.tile([P, D], f16, tag=f"ow{i}", name=f"ow{i}") for i in range(ET)]
            def load_out_w():
                for i in range(ET):
                    nc.sync.dma_start(out_w[i][:], out_wT[128 * i:128 * (i + 1), :])
            xp_w = [pers.tile([P, R + 2 * N], f16, tag=f"xpw{i}", name=f"xpw{i}") for i in range(ET)]
            for i in range(ET):
                nc.sync.dma_start(xp_w[i][:], xp_wT[128 * i:128 * (i + 1), :])
            dtp_w = pers.tile([R, E], f16, tag="dtpw", name="dtpw")
            nc.sync.dma_start(dtp_w[:], dtp_wT[:])
            cw = pers.tile([P, ET * KC], f32, tag="cw", name="cw")
            nc.sync.dma_start(cw[:], convw[:])
            cb = pers.tile([P, ET], f32, tag="cb", name="cb")
            nc.sync.dma_start(cb[:], convb[:])
            dtbt = pers.tile([P, ET], f32, tag="dtb", name="dtb")
            nc.sync.dma_start(dtbt[:], dtb[:])
            dvt = pers.tile([P, ET], f32, tag="dv", name="dv")
            nc.sync.dma_start(dvt[:], dvec[:])
            # A = -exp(A_log) precomputed on host
            At = pers.tile([P, ET * N], f32, tag="A", name="A")
            nc.sync.dma_start(At[:], avals[:])
            nwt = pers.tile([P, DT], f32, tag="nw", name="nw")
            nc.sync.dma_start(nwt[:], nw[:])
            nbt = pers.tile([P, DT], f32, tag="nb", name="nb")
            nc.sync.dma_start(nbt[:], nb[:])

            ones = pers.tile([P, 1], f16, tag="ones", name="ones")
            nc.vector.memset(ones[:], 1.0)
            from concourse import masks
            ident = pers.tile([P, P], f16, tag="ident", name="ident")
            masks.make_identity(nc, ident[:])
            epst = pers.tile([P, 1], f32, tag="epst", name="epst")
            nc.vector.memset(epst[:], EPS)

            # depthwise-conv taps as diagonal matrices (PE lhsT)
            dgw = [[pers.tile([P, P], f16, tag=f"dg{i}_{k}", name=f"dg{i}_{k}")
                    for k in range(KC)] for i in range(ET)]
            for i in range(ET):
                for k in range(KC):
                    nc.vector.tensor_scalar_mul(dgw[i][k][:], ident[:],
                                                cw[:, KC * i + k:KC * i + k + 1])

            # D-vector as diagonal matrices for the y-seed matmul
            ddv = [pers.tile([P, P], f16, tag=f"ddv{i}", name=f"ddv{i}") for i in range(ET)]
            for i in range(ET):
                nc.vector.tensor_scalar_mul(ddv[i][:], ident[:], dvt[:, i:i + 1])

            # scan carry state h[:, (i,n)] and conv tails
            carry = [pers.tile([P, N], f16, tag=f"carry{i}", name=f"carry{i}") for i in range(ET)]
            xtail = [pers.tile([P, 3], f16, tag=f"xtail{i}", name=f"xtail{i}") for i in range(ET)]
            for i in range(ET):
                nc.vector.memset(xtail[i][:], 0.0)

            # ---- streaming pools ----
            hst_p = pool("hst", 1)     # hsT chunk tiles (f16)
            sq_p = pool("sq", 2)       # squared sub-tiles (f16, transient)
            srow_p = pool("srow", 1)   # stat rows (1, TL) f16
            rep_p = pool("rep", 1)     # broadcast stat rows (128, TL) f16
            hn_p = pool("hn", 1)       # normalized hs (f16), DT tags
            xpad_p = pool("xpad", 3)   # conv input [3 | TL] f16, cycled
            xc_p = pool("xc", 1)       # conv output f16, ET tags
            z_p = pool("z", 2)         # z spill bounce (small)
            xdbl_p = pool("xdbl", 2)   # (64, TL) f16
            esp_p = pool("esp", 1)     # exp(dt_raw) staging f32, DTG tags
            dl_p = pool("dl", 1)       # delta f16, ET tags (fully resident)
            du_p = pool("du", 1)       # delta*u f16, ET tags (fully resident)
            y_p = pool("y", 1)         # y gated f16, ET tags
            bc_p = pool("bc", 2)       # B/C broadcast planes f16
            tr_p = pool("tr", 2)       # scan transients f16
            ov_p = pool("ov", 1)       # out_proj result f16
            tmp_p = pool("tmp", 1)     # misc small

            import itertools

            chunks = [(rep, c) for rep in range(reps) for c in range(NL)]
            NC = len(chunks)
            states = [dict() for _ in range(NC)]

            def emit_LN(ci):
                st = states[ci]
                lo = chunks[ci][1] * TL
                hst = []
                for k in range(DT):
                    t = hst_p.tile([P, TL], f16, tag=f"hst{k}", name=f"hst{k}")
                    nc.sync.dma_start(t[:], hsT[128 * k:128 * (k + 1), lo:lo + TL])
                    hst.append(t)
                mu = srow_p.tile([1, TL], f16, tag="mu", name="mu")
                msq = srow_p.tile([1, TL], f16, tag="msq", name="msq")
                for s in range(NSUB):
                    sl = slice(512 * s, 512 * (s + 1))
                    st_ps = ps_aux.tile([33, 512], f32, tag="aux", name="stps",
                                        padded_shape=[128, 512])
                    mu_ps, sq_ps = st_ps[0:1, :], st_ps[32:33, :]
                    for k in range(DT):
                        sqt = sq_p.tile([P, 512], f16, tag="sqt", name="sqt")
                        nc.vector.tensor_tensor(sqt[:], hst[k][:, sl], hst[k][:, sl], op=op.mult)
                        nc.tensor.matmul(mu_ps, ones[:], hst[k][:, sl], skip_group_check=True,
                                         start=(k == 0), stop=(k == DT - 1))
                        nc.tensor.matmul(sq_ps, ones[:], sqt[:], skip_group_check=True,
                                         start=(k == 0), stop=(k == DT - 1))
                    nc.scalar.activation(mu[:, sl], mu_ps, AF.Copy, scale=1.0 / D)
                    nc.scalar.activation(msq[:, sl], sq_ps, AF.Copy, scale=1.0 / D)
                mu2 = srow_p.tile([1, TL], f16, tag="mu2", name="mu2")
                nc.vector.tensor_tensor(mu2[:], mu[:], mu[:], op=op.mult)
                nc.vector.tensor_sub(msq[:], msq[:], mu2[:])
                nc.scalar.activation(msq[:], msq[:], AF.Sqrt, bias=epst[0:1, :])
                with nc.allow_low_precision(reason="rsqrt row in f16; |rs|~1"):
                    nc.vector.reciprocal(mu2[:], msq[:])
                murep = rep_p.tile([P, TL], f16, tag="murep", name="murep")
                nc.gpsimd.partition_broadcast(murep[:], mu[:])
                rsrep = rep_p.tile([P, TL], f16, tag="rsrep", name="rsrep")
                nc.gpsimd.partition_broadcast(rsrep[:], mu2[:])
                hn = []
                for k in range(DT):
                    rsw = tmp_p.tile([P, TL], f16, tag="rsw", name="rsw")
                    nc.vector.tensor_scalar_mul(rsw[:], rsrep[:], nwt[:, k:k + 1])
                    bias2 = tmp_p.tile([P, TL], f16, tag="bias2", name="bias2")
                    nc.vector.tensor_tensor(bias2[:], murep[:], rsw[:], op=op.mult)
                    nc.vector.tensor_scalar(bias2[:], bias2[:], -1.0, nbt[:, k:k + 1],
                                            op0=op.mult, op1=op.add)
                    ht = hn_p.tile([P, TL], f16, tag=f"hn{k}", name=f"hn{k}")
                    nc.vector.tensor_tensor(ht[:], hst[k][:], rsw[:], op=op.mult)
                    nc.vector.tensor_add(ht[:], ht[:], bias2[:])
                    hn.append(ht)
                st["hn"] = hn
                st["xpads"] = []
                st["xcs"] = []
                st["xcps"] = []
                st["dls"] = []
                st["dus"] = []
                st["dlp"] = []
                st["dup"] = []
                st["esps"] = {}

            def emit_inproj_m(ci, m):
                st = states[ci]
                hn = st["hn"]
                xp = xpad_p.tile([P, TL + 3], f16, tag="xpad", name="xpad")
                nc.scalar.copy(xp[:, 0:3], xtail[m][:])
                st["xpads"].append(xp)
                for s in range(NSUB):
                    sl = slice(512 * s, 512 * (s + 1))
                    ps = ps_pool.tile([P, 512], f32, tag="mm", name="mm")
                    for k in range(DT):
                        nc.tensor.matmul(ps[:],
                                         w_in[k][:, 128 * m:128 * (m + 1)],
                                         hn[k][:, sl],
                                         start=(k == 0), stop=(k == DT - 1))
                    nc.scalar.copy(xp[:, 3 + 512 * s:3 + 512 * (s + 1)], ps[:])

            def emit_z(ci, mz):
                st = states[ci]
                hn = st["hn"]
                lo_par = ci % 2
                m = ET + mz
                zt = z_p.tile([P, TL], f16, tag="zsp", name="zsp")
                for s in range(NSUB):
                    sl = slice(512 * s, 512 * (s + 1))
                    ps = ps_pool.tile([P, 512], f32, tag="mm", name="mm")
                    for k in range(DT):
                        nc.tensor.matmul(ps[:],
                                         w_in[k][:, 128 * m:128 * (m + 1)],
                                         hn[k][:, sl],
                                         start=(k == 0), stop=(k == DT - 1))
                    nc.scalar.copy(zt[:, sl], ps[:])
                nc.sync.dma_start(zdram[lo_par][128 * mz:128 * (mz + 1), :], zt[:])

            def emit_conv(ci, i):
                st = states[ci]
                xp = st["xpads"][i]
                nc.scalar.copy(xtail[i][:], xp[:, TL:TL + 3])
                if merged:
                    if i % 2 == 0:
                        xcp = xc_p.tile([P, 2 * TL], f16, tag=f"xcp{i // 2}",
                                        name=f"xcp{i // 2}",
                                        bufs=2 if i >= 6 else 1)
                        st["xcps"].append(xcp)
                    xcp = st["xcps"][i // 2]
                    xct = xcp[:, (i % 2) * TL:(i % 2 + 1) * TL]
                else:
                    xct = xc_p.tile([P, TL], f16, tag=f"xc{i}", name=f"xc{i}",
                                    bufs=2 if i >= 6 else 1)
                for s in range(NSUB):
                    sl = slice(512 * s, 512 * (s + 1))
                    psc = ps_pool.tile([P, 512], f32, tag="mm", name="mmc")
                    for k in range(KC):
                        nc.tensor.matmul(psc[:], dgw[i][k][:],
                                         xp[:, 512 * s + k:512 * s + k + 512],
                                         start=(k == 0), stop=(k == KC - 1))
                    nc.scalar.activation(xct[:, sl], psc[:], AF.Silu, bias=cb[:, i:i + 1])
                st["xcs"].append(xct)

            def emit_xproj(ci):
                st = states[ci]
                xcs = st["xcs"]
                xdbl = xdbl_p.tile([R, TL], f16, tag="xdbl", name="xdbl")
                bcs = xdbl_p.tile([2 * N, TL], f16, tag="bcs", name="bcs", bufs=1)
                for s in range(NSUB):
                    sl = slice(512 * s, 512 * (s + 1))
                    ps = ps_aux.tile([R + 2 * N, 512], f32, tag="aux", name="xdblps",
                                     padded_shape=[128, 512])
                    for i in range(ET):
                        nc.tensor.matmul(ps[:], xp_w[i][:], xcs[i][:, sl],
                                         start=(i == 0), stop=(i == ET - 1))
                    nc.scalar.copy(xdbl[:, sl], ps[0:R, :])
                    nc.scalar.copy(bcs[:, sl], ps[R:R + 2 * N, :])
                nc.sync.dma_start(bcd[ci % 2][:, :], bcs[:])
                st["xdbl"] = xdbl

            def emit_dt_exps(ci, g):
                st = states[ci]
                xdbl = st["xdbl"]
                if merged:
                    espp = esp_p.tile([P, 2 * TL], f16, tag="espp", name=f"espp{g}")
                    st["esps"][g] = espp
                for ii in range(DTG):
                    i = g * DTG + ii
                    if merged:
                        esp = st["esps"][g][:, ii * TL:(ii + 1) * TL]
                    else:
                        esp = esp_p.tile([P, TL], f16, tag=f"esp{ii}", name=f"esp{i}")
                        st["esps"][i] = esp
                    for s in range(NSUB):
                        sl = slice(512 * s, 512 * (s + 1))
                        ps = ps_pool.tile([P, 512], f32, tag="mm", name="mm")
                        nc.tensor.matmul(ps[:], dtp_w[:, 128 * i:128 * (i + 1)],
                                         xdbl[0:R, sl], start=True, stop=True)
                        nc.scalar.activation(esp[:, sl], ps[:], AF.Exp,
                                             bias=dtbt[:, i:i + 1])

            def emit_dt_ln_du(ci, i):
                # merged: i is a pair index g (2 e-tiles); else a single e-tile
                st = states[ci]
                if merged:
                    g = i
                    dlp = dl_p.tile([P, 2 * TL], f16, tag=f"dlp{g}", name=f"dlp{g}")
                    nc.scalar.activation(dlp[:], st["esps"][g][:], AF.Ln, bias=1.0)
                    st["dlp"].append(dlp)
                    st["dls"].extend([dlp[:, 0:TL], dlp[:, TL:2 * TL]])
                    dup = du_p.tile([P, 2 * TL], f16, tag=f"dup{g}", name=f"dup{g}")
                    nc.vector.tensor_tensor(dup[:], dlp[:], st["xcps"][g][:], op=op.mult)
                    st["dup"].append(dup)
                    st["dus"].extend([dup[:, 0:TL], dup[:, TL:2 * TL]])
                else:
                    dl = dl_p.tile([P, TL], f16, tag=f"dl{i}", name=f"dl{i}")
                    nc.scalar.activation(dl[:], st["esps"][i][:], AF.Ln, bias=1.0)
                    st["dls"].append(dl)
                    du = du_p.tile([P, TL], f16, tag=f"du{i}", name=f"du{i}")
                    nc.vector.tensor_tensor(du[:], dl[:], st["xcs"][i][:], op=op.mult)
                    st["dus"].append(du)

            def emit_gate(ci, i):
                st = states[ci]
                zr = z_p.tile([P, TL], f16, tag="zr", name="zr")
                nc.sync.dma_start(zr[:], zdram[ci % 2][128 * i:128 * (i + 1), :])
                zst = tmp_p.tile([P, TL], f16, tag="zs", name="zs")
                nc.scalar.activation(zst[:], zr[:], AF.Silu)
                nc.vector.tensor_tensor(st["yfin"][i][:], st["yfin"][i][:], zst[:], op=op.mult)

            def emit_outproj(ci, m):
                st = states[ci]
                lo = chunks[ci][1] * TL
                ygs = st["yfin"]
                ov = ov_p.tile([P, TL], f16, tag="ov", name="ov")
                for s in range(NSUB):
                    sl = slice(512 * s, 512 * (s + 1))
                    ps = ps_pool.tile([P, 512], f32, tag="mm", name="mm")
                    for i in range(ET):
                        nc.tensor.matmul(ps[:],
                                         out_w[i][:, 128 * m:128 * (m + 1)],
                                         ygs[i][:, sl],
                                         start=(i == 0), stop=(i == ET - 1))
                    nc.scalar.copy(ov[:, sl], ps[:])
                nc.sync.dma_start(y_part[128 * m:128 * (m + 1), lo:lo + TL], ov[:])

            def head_schedule(ci, shift=0):
                """Pacing of chunk ci's head into chunk ci-1's scan loop.
                Keys are scan iteration index it = ib*16 + n (0..63)."""
                S = {}
                def sh(d):
                    return {min(k + shift, 63): v for k, v in d.items()}
                S[1] = [lambda: emit_LN(ci)]
                if ci == 1:
                    S[1] = S[1] + [load_out_w]
                S[2] = [lambda: emit_inproj_m(ci, 0)]
                S[3] = [lambda: emit_inproj_m(ci, 1)]
                S[4] = [lambda: emit_inproj_m(ci, 2)]
                S[6] = [lambda: emit_conv(ci, 0), lambda: emit_conv(ci, 1)]
                S[7] = [lambda: emit_inproj_m(ci, 3)]
                S[8] = [lambda: emit_inproj_m(ci, 4)]
                for j in range(4):
                    S[9 + j] = [lambda j=j: emit_z(ci, j)]
                S[17] = [lambda: emit_conv(ci, 2), lambda: emit_conv(ci, 3)]
                S[18] = [lambda: emit_inproj_m(ci, 5)]
                S[19] = [lambda: emit_inproj_m(ci, 6)]
                for j in range(4):
                    S[20 + j] = [lambda j=j: emit_z(ci, 4 + j)]
                S[33] = [lambda: emit_conv(ci, 4), lambda: emit_conv(ci, 5)]
                S[34] = [lambda: emit_inproj_m(ci, 7)]
                S[35] = [lambda: emit_conv(ci, 6), lambda: emit_conv(ci, 7)]
                S[36] = [lambda: emit_xproj(ci)]
                S[37] = [lambda: emit_dt_exps(ci, 0)]
                S[40] = [lambda: emit_dt_exps(ci, 1)]
                S[43] = [lambda: emit_dt_exps(ci, 2)]
                if merged:
                    S[50] = [lambda: emit_dt_exps(ci, 3)]
                    S[39] = [lambda: emit_dt_ln_du(ci, 0)]
                    S[42] = [lambda: emit_dt_ln_du(ci, 1)]
                    S[49] = [lambda: emit_dt_ln_du(ci, 2)]
                    leftover = [lambda: emit_dt_ln_du(ci, 3)]
                else:
                    S[44] = [lambda: emit_dt_exps(ci, 3)]
                    S[39] = [lambda: emit_dt_ln_du(ci, 0), lambda: emit_dt_ln_du(ci, 1)]
                    S[42] = [lambda: emit_dt_ln_du(ci, 2), lambda: emit_dt_ln_du(ci, 3)]
                    S[49] = [lambda: emit_dt_ln_du(ci, 4), lambda: emit_dt_ln_du(ci, 5)]
                    leftover = [lambda: emit_dt_ln_du(ci, 6), lambda: emit_dt_ln_du(ci, 7)]
                if shift:
                    shifted = {}
                    for k, v in S.items():
                        shifted.setdefault(min(k + shift, 61), []).extend(v)
                    S = shifted
                return S, leftover

            def tail_schedule(ci):
                """Pacing of chunk ci's tail (gate + out_proj) into scan(ci+1)."""
                S = {}
                S[0] = [lambda i=i: emit_gate(ci, i) for i in range(ET)]
                for m in range(DT):
                    S.setdefault(8 + 2 * m, []).append(lambda m=m: emit_outproj(ci, m))
                return S

            def emit_scan(ci, sched, inline_gate=False):
                st = states[ci]
                c = chunks[ci][1]
                dls, dus, xcs = st["dls"], st["dus"], st["xcs"]
                ypss, yfin = {}, {}
                st["yfin"] = yfin
                for ib, n in itertools.product(range(ET // BLK), range(N)):
                    it = ib * N + n
                    if merged:
                        W = 2 * TL
                        bp = bc_p.tile([P, TL], f16, tag="bp", name="bp")
                        nc.sync.dma_start(bp[:], bcd[ci % 2][n:n + 1, :].to_broadcast((P, TL)))
                        cp = bc_p.tile([P, TL], f16, tag="cp", name="cp")
                        nc.sync.dma_start(cp[:], bcd[ci % 2][N + n:N + n + 1, :].to_broadcast((P, TL)))
                        i0 = BLK * ib
                        if n == 0:
                            for i in range(i0, i0 + BLK):
                                yp = ps_y.tile([P, TL], f32, tag="yps", name="yps")
                                ypss[i] = yp
                                for sb in range(NSUB):
                                    sl = slice(512 * sb, 512 * (sb + 1))
                                    nc.tensor.matmul(yp[:, sl], ddv[i][:], xcs[i][:, sl],
                                                     start=True, stop=False)
                        da = tr_p.tile([P, W], f16, tag="da", name="da", bufs=2)
                        nc.scalar.activation(da[:], st["dlp"][ib][:], AF.Exp,
                                             scale=At[:, N * i0 + n:N * i0 + n + 1])
                        db = tr_p.tile([P, W], f16, tag="db", name="db", bufs=2)
                        for h in range(2):
                            hsl = slice(h * TL, (h + 1) * TL)
                            if ((i0 + h) * N + n) % 6 == 0:
                                nc.vector.tensor_tensor(db[:, hsl], st["dup"][ib][:, hsl],
                                                        bp[:], op=op.mult)
                            else:
                                nc.gpsimd.tensor_tensor(db[:, hsl], st["dup"][ib][:, hsl],
                                                        bp[:], op=op.mult)
                        if c > 0:
                            # inject e-tile i0+1's carry through the boundary column
                            nc.vector.scalar_tensor_tensor(
                                db[:, TL:TL + 1], da[:, TL:TL + 1],
                                carry[i0 + 1][:, n:n + 1], db[:, TL:TL + 1],
                                op0=op.mult, op1=op.add)
                        nc.vector.memset(da[:, TL:TL + 1], 0.0)
                        hh = tr_p.tile([P, W], f16, tag="hh", name="hh", bufs=2)
                        init = 0.0 if c == 0 else carry[i0][:, n:n + 1]
                        nc.vector.tensor_tensor_scan(hh[:], da[:], db[:], init,
                                                     op0=op.mult, op1=op.add)
                        gg = tr_p.tile([P, W], f16, tag="gg", name="gg", bufs=2)
                        for h in range(2):
                            hsl = slice(h * TL, (h + 1) * TL)
                            nc.vector.tensor_tensor(gg[:, hsl], hh[:, hsl], cp[:], op=op.mult)
                        if c < NL - 1:
                            for h in range(2):
                                nc.vector.tensor_copy(carry[i0 + h][:, n:n + 1],
                                                      hh[:, (h + 1) * TL - 1:(h + 1) * TL])
                        for i in range(i0, i0 + BLK):
                            off = (i - i0) * TL
                            for sb in range(NSUB):
                                sl = slice(off + 512 * sb, off + 512 * (sb + 1))
                                osl = slice(512 * sb, 512 * (sb + 1))
                                nc.tensor.matmul(ypss[i][:, osl], ident[:], gg[:, sl],
                                                 start=False, stop=(n == N - 1))
                            if n == N - 1:
                                yt = y_p.tile([P, TL], f16, tag=f"y{i}", name=f"y{i}")
                                nc.scalar.copy(yt[:], ypss[i][:])
                                yfin[i] = yt
                                if inline_gate:
                                    emit_gate(ci, i)
                        for thunk in sched.get(it, ()):
                            thunk()
                        continue
                    bp = bc_p.tile([P, TL], f16, tag="bp", name="bp")
                    nc.sync.dma_start(bp[:], bcd[ci % 2][n:n + 1, :].to_broadcast((P, TL)))
                    cp = bc_p.tile([P, TL], f16, tag="cp", name="cp")
                    nc.sync.dma_start(cp[:], bcd[ci % 2][N + n:N + n + 1, :].to_broadcast((P, TL)))
                    for i in range(BLK * ib, BLK * (ib + 1)):
                        if n == 0:
                            yp = ps_y.tile([P, TL], f32, tag="yps", name="yps")
                            ypss[i] = yp
                            for sb in range(NSUB):
                                sl = slice(512 * sb, 512 * (sb + 1))
                                nc.tensor.matmul(yp[:, sl], ddv[i][:], xcs[i][:, sl],
                                                 start=True, stop=False)
                        da = tr_p.tile([P, TL], f16, tag="da", name="da", bufs=3)
                        nc.scalar.activation(da[:], dls[i][:], AF.Exp,
                                             scale=At[:, N * i + n:N * i + n + 1])
                        db = tr_p.tile([P, TL], f16, tag="db", name="db", bufs=3)
                        if (i * N + n) % 7 == 0:
                            nc.vector.tensor_tensor(db[:], dus[i][:], bp[:], op=op.mult)
                        else:
                            nc.gpsimd.tensor_tensor(db[:], dus[i][:], bp[:], op=op.mult)
                        hh = tr_p.tile([P, TL], f16, tag="hh", name="hh", bufs=3)
                        init = 0.0 if c == 0 else carry[i][:, n:n + 1]
                        nc.vector.tensor_tensor_scan(hh[:], da[:], db[:], init,
                                                     op0=op.mult, op1=op.add)
                        gg = tr_p.tile([P, TL], f16, tag="gg", name="gg", bufs=3)
                        nc.vector.tensor_tensor(gg[:], hh[:], cp[:], op=op.mult)
                        if c < NL - 1:
                            nc.vector.tensor_copy(carry[i][:, n:n + 1], hh[:, TL - 1:TL])
                        for sb in range(NSUB):
                            sl = slice(512 * sb, 512 * (sb + 1))
                            nc.tensor.matmul(ypss[i][:, sl], ident[:], gg[:, sl],
                                             start=False, stop=(n == N - 1))
                        if n == N - 1:
                            yt = y_p.tile([P, TL], f16, tag=f"y{i}", name=f"y{i}")
                            nc.scalar.copy(yt[:], ypss[i][:])
                            yfin[i] = yt
                            if inline_gate:
                                emit_gate(ci, i)
                    for thunk in sched.get(it, ()):
                        thunk()

            # ---- software-pipelined emission over all chunks ----
            # chunk 0: emit only the critical head path before scan(0); the
            # z half and the late dt groups are paced into scan(0) itself.
            emit_LN(0)
            for m in (0, 1, 2):
                emit_inproj_m(0, m)
            emit_conv(0, 0); emit_conv(0, 1)
            for m in (3, 4):
                emit_inproj_m(0, m)
            emit_conv(0, 2); emit_conv(0, 3)
            for m in (5, 6):
                emit_inproj_m(0, m)
            emit_conv(0, 4); emit_conv(0, 5)
            emit_inproj_m(0, 7)
            emit_conv(0, 6); emit_conv(0, 7)
            emit_xproj(0)
            emit_dt_exps(0, 0)
            if merged:
                emit_dt_ln_du(0, 0)
                head0_rest = {1: [lambda: emit_dt_exps(0, 1)],
                              2: [lambda: emit_dt_ln_du(0, 1)],
                              4: [lambda: emit_dt_exps(0, 2)],
                              5: [lambda: emit_dt_ln_du(0, 2)],
                              7: [lambda: emit_dt_exps(0, 3)],
                              8: [lambda: emit_dt_ln_du(0, 3)]}
            else:
                emit_dt_ln_du(0, 0); emit_dt_ln_du(0, 1)
                head0_rest = {1: [lambda: emit_dt_exps(0, 1)],
                              2: [lambda: emit_dt_ln_du(0, 2), lambda: emit_dt_ln_du(0, 3)],
                              4: [lambda: emit_dt_exps(0, 2)],
                              5: [lambda: emit_dt_ln_du(0, 4), lambda: emit_dt_ln_du(0, 5)],
                              7: [lambda: emit_dt_exps(0, 3)],
                              8: [lambda: emit_dt_ln_du(0, 6), lambda: emit_dt_ln_du(0, 7)]}
            for j in range(8):
                head0_rest.setdefault(3 + j, []).append(lambda j=j: emit_z(0, j))
            for ci in range(NC):
                sched = {}
                if ci == 0:
                    for it, ts in head0_rest.items():
                        sched.setdefault(it, []).extend(ts)
                if ci > 0:
                    for it, ts in tail_schedule(ci - 1).items():
                        sched.setdefault(it, []).extend(ts)
                leftover = []
                if ci + 1 < NC:
                    hS, leftover = head_schedule(ci + 1, shift=12 if ci == 0 else 0)
                    for it, ts in hS.items():
                        sched.setdefault(it, []).extend(ts)
                emit_scan(ci, sched, inline_gate=(ci == NC - 1))
                for thunk in leftover:
                    thunk()
            for m in range(DT):
                emit_outproj(NC - 1, m)

    nc.compile()
    return nc


def _build_stage_b(reps=1):
    import concourse.tile as tile
    from concourse import bacc, mybir
    from concourse.alu_op_type import AluOpType as op
    from contextlib import ExitStack

    dt = mybir.dt
    f32, f16 = dt.float32, dt.float16
    AF = mybir.ActivationFunctionType
    LH = L // 2  # 1024 rows per core

    nc = bacc.Bacc("TRN2", target_bir_lowering=False, debug=False, num_devices=8)
    yf = nc.dram_tensor("yf", [LH, D], f16, kind="ExternalInput").ap()
    yr = nc.dram_tensor("yr", [LH, D], f16, kind="ExternalInput").ap()
    res = nc.dram_tensor("res", [LH, D], f16, kind="ExternalInput").ap()
    nfw = nc.dram_tensor("nfw", [1, D], f32, kind="ExternalInput").ap()
    out = nc.dram_tensor("out", [LH, D], f32, kind="ExternalOutput").ap()

    with tile.TileContext(nc) as tc:
        with ExitStack() as ctx:
            P = 128
            pers = ctx.enter_context(tc.tile_pool(name="pers", bufs=1))
            io_p = ctx.enter_context(tc.tile_pool(name="io", bufs=4))
            tmp_p = ctx.enter_context(tc.tile_pool(name="tmp", bufs=4))

            epst = pers.tile([128, 1], f32, tag="epst", name="epst")
            nc.vector.memset(epst[:], EPS)
            nfwt = pers.tile([1, D], f32, tag="nfw", name="nfw")
            nc.sync.dma_start(nfwt[:], nfw[:])
            nfr = pers.tile([P, D], f32, tag="nfr", name="nfr")
            nc.gpsimd.partition_broadcast(nfr[:], nfwt[:])

            import itertools
            for rep, t in itertools.product(range(reps), range(LH // P)):
                rows = slice(P * t, P * (t + 1))
                tf = io_p.tile([P, D], f16, tag="tf", name="tf")
                nc.sync.dma_start(tf[:], yf[rows, :])
                tr = io_p.tile([P, D], f16, tag="tr", name="tr")
                nc.sync.dma_start(tr[:], yr[rows, :])
                tres = io_p.tile([P, D], f16, tag="tres", name="tres")
                nc.sync.dma_start(tres[:], res[rows, :])
                s = tmp_p.tile([P, D], f32, tag="s", name="s")
                nc.vector.tensor_add(s[:], tf[:], tr[:])
                nc.vector.tensor_add(s[:], s[:], tres[:])
                sq = tmp_p.tile([P, D], f32, tag="sq", name="sq")
                ssum = tmp_p.tile([P, 1], f32, tag="ssum", name="ssum")
                nc.scalar.activation(sq[:], s[:], AF.Square, accum_out=ssum[:])
                lnm = tmp_p.tile([P, 1], f32, tag="lnm", name="lnm")
                nc.scalar.activation(lnm[:], ssum[:], AF.Sqrt, bias=epst[:], scale=1.0 / D)
                rinv = tmp_p.tile([P, 1], f32, tag="rinv", name="rinv")
                nc.vector.reciprocal(rinv[:], lnm[:])
                o = tmp_p.tile([P, D], f32, tag="o", name="o")
                nc.vector.scalar_tensor_tensor(o[:], s[:], rinv[:], nfr[:],
                                               op0=op.mult, op1=op.mult)
                nc.sync.dma_start(out[rows, :], o[:])

    nc.compile()
    return nc


class _Runner:
    """Compile a Bass program once into a sharded PJRT callable for 8 cores."""

    def __init__(self, nc, n_cores=8):
        import jax
        import jax.numpy as jnp
        from jax.sharding import Mesh, PartitionSpec
        from jax.experimental.shard_map import shard_map
        from concourse import bass2jax, mybir

        bass2jax.install_neuronx_cc_hook()
        self.n_cores = n_cores
        in_names, out_names, out_avals, zero_outs = [], [], [], []
        partition_name = nc.partition_id_tensor.name if nc.partition_id_tensor else None
        for alloc in nc.m.functions[0].allocations:
            if not isinstance(alloc, mybir.MemoryLocationSet):
                continue
            name = alloc.memorylocations[0].name
            if alloc.kind == "ExternalInput":
                if name != partition_name:
                    in_names.append(name)
            elif alloc.kind == "ExternalOutput":
                shape = tuple(alloc.tensor_shape)
                dtype = mybir.dt.np(alloc.dtype)
                out_names.append(name)
                out_avals.append(jax.core.ShapedArray(shape, dtype))
                zero_outs.append(np.zeros((n_cores * shape[0],) + shape[1:], dtype))
        self.in_names, self.out_names, self.out_avals = in_names, out_names, out_avals
        n_params, n_outs = len(in_names), len(out_names)
        all_names = list(in_names) + list(out_names)
        if partition_name is not None:
            all_names.append(partition_name)

        def _body(*args):
            operands = list(args)
            if partition_name is not None:
                operands.append(bass2jax.partition_id_tensor())
            outs = bass2jax._bass_exec_p.bind(
                *operands,
                out_avals=tuple(out_avals),
                in_names=tuple(all_names),
                out_names=tuple(out_names),
                lowering_input_output_aliases=(),
                sim_require_finite=True,
                sim_require_nnan=True,
                nc=nc,
            )
            return tuple(outs)

        devices = jax.devices()[:n_cores]
        mesh = Mesh(np.asarray(devices), ("core",))
        in_specs = (PartitionSpec("core"),) * (n_params + n_outs)
        out_specs = (PartitionSpec("core"),) * n_outs
        self.fn = jax.jit(
            shard_map(_body, mesh=mesh, in_specs=in_specs,
                      out_specs=out_specs, check_rep=False),
            keep_unused=True)
        self.mesh = mesh
        self._zero_dev = [jax.device_put(z) for z in zero_outs]

    def prep(self, in_maps):
        import jax
        assert len(in_maps) == self.n_cores
        concat = [np.concatenate([np.asarray(m[n]) for m in in_maps], axis=0)
                  for n in self.in_names]
        return [jax.device_put(a) for a in concat] + self._zero_dev

    def run_dev(self, dev_args):
        return self.fn(*dev_args)

    def __call__(self, in_maps):
        import jax
        out_arrs = self.fn(*self.prep(in_maps))
        out_arrs = [np.asarray(a) for a in out_arrs]
        res = []
        for c in range(self.n_cores):
            d = {}
            for i, name in enumerate(self.out_names):
                shape = self.out_avals[i].shape
                d[name] = out_arrs[i].reshape((self.n_cores,) + shape)[c]
            res.append(d)
        return res


def _programs(merged=True):
    key = ("a", merged)
    if key not in _cache:
        _cache[key] = _Runner(_build_stage_a(merged=merged))
        _cache["a"] = _cache[key]
    if "b" not in _cache:
        _cache["b"] = _Runner(_build_stage_b())
    return _cache[key], _cache["b"]


def _pack_cols(v, width):
    # (E,)-like flat -> (128, ET*width) per-partition column blocks
    a = np.asarray(v, np.float32).reshape(ET, 128, width)
    return np.ascontiguousarray(a.transpose(1, 0, 2).reshape(128, ET * width))


def kernel(**inputs):
    # The 2048-wide merged scan applies one per-partition A column to both
    # e-tiles of a block; valid only when A is constant across channels
    # (true for the standard Mamba init A_log = log(1..N) tiled). Verify on
    # the actual inputs and fall back to the unmerged build otherwise.
    merged = True
    for nm in ("A_log", "A_b_log"):
        a = -np.exp(np.asarray(inputs[nm], np.float32))
        if not np.allclose(a, a[0:1, :], rtol=1e-6, atol=1e-7):
            merged = False
    run_a, run_b = _programs(merged)
    f16 = np.float16
    hs = np.asarray(inputs["hidden_states"], np.float32)

    w_inT = np.ascontiguousarray(np.asarray(inputs["in_proj_w"], np.float32).T).astype(f16)
    out_wT = np.ascontiguousarray(np.asarray(inputs["out_proj_w"], np.float32).T).astype(f16)
    # norm_w/b are per-D; in (D,L) layout D is the partition dim -> column k = rows 128k..128k+127
    nw = np.ascontiguousarray(np.asarray(inputs["norm_w"], np.float32).reshape(DT, 128).T)
    nb = np.ascontiguousarray(np.asarray(inputs["norm_b"], np.float32).reshape(DT, 128).T)

    per_dir = {}
    for d, sfx in ((0, ""), (1, "_b")):
        alog = np.asarray(inputs["A_log" if d == 0 else "A_b_log"], np.float32)
        per_dir[d] = dict(
            xp_wT=np.ascontiguousarray(np.asarray(inputs["x_proj_w" + sfx], np.float32).T).astype(f16),
            dtp_wT=np.ascontiguousarray(np.asarray(inputs["dt_proj_w" + sfx], np.float32).T).astype(f16),
            convw=_pack_cols(inputs["conv_w" + sfx], KC),
            convb=_pack_cols(inputs["conv_b" + sfx], 1),
            dtb=_pack_cols(inputs["dt_proj_b" + sfx], 1),
            avals=_pack_cols(-np.exp(alog), N),
            dvec=_pack_cols(inputs["D_fwd" if d == 0 else "D_bwd"], 1),
        )

    in_maps = []
    for c in range(8):
        b, d = c % 4, c // 4
        h = hs[b] if d == 0 else hs[b, ::-1]
        in_maps.append(dict(
            hsT=np.ascontiguousarray(h.T).astype(f16),
            w_inT=w_inT, out_wT=out_wT, nw=nw, nb=nb,
            **per_dir[d],
        ))
    _cache["last_in_maps_a"] = in_maps
    res_a = run_a(in_maps)

    LH = L // 2
    nfw = np.asarray(inputs["normf_w"], np.float32).reshape(1, D)
    in_maps_b = []
    for c in range(8):
        b, half = c % 4, c // 4
        rows = slice(half * LH, (half + 1) * LH)
        yfT = res_a[b]["y_part"].T            # (L, D) f16
        yrT = res_a[b + 4]["y_part"][:, ::-1].T
        in_maps_b.append(dict(
            yf=np.ascontiguousarray(yfT[rows]),
            yr=np.ascontiguousarray(yrT[rows]),
            res=np.ascontiguousarray(hs[b, rows]).astype(f16),
            nfw=nfw,
        ))
    _cache["last_in_maps_b"] = in_maps_b
    res_b = run_b(in_maps_b)

    out = np.empty((B, L, D), np.float32)
    for c in range(8):
        b, half = c % 4, c // 4
        out[b, half * LH:(half + 1) * LH] = res_b[c]["out"]
    return out


# revision 50
# speedup vs baseline: 1.0098x; 1.0098x over previous
"""Bidirectional Mamba block on 8 Trainium2 NeuronCores.

Sharding: core c in 0..7 handles (batch b = c % 4, direction d = c // 4).
The two directions of one batch are independent branches until the final
out_proj-sum + residual + RMSNorm, which a second tiny SPMD kernel does
(8 cores = 4 batches x 2 sequence halves).

Stage A (per core), per 1024-column chunk: LayerNorm (PE ones-matmul stats,
f16 rows, sqrt+reciprocal rsqrt) -> in_proj -> causal dwconv as 4 PE
diag(w_k)-matmuls accumulated in PSUM + SiLU-from-PSUM -> x_proj ->
dt_proj+softplus (exp batched ahead of ln to limit activation-table
reloads) -> selective scan: the two e-tiles of each block are fused into
one 2048-wide recurrence (single da exp on ScalarE, single
tensor_tensor_scan on DVE; the segment boundary column of da is zeroed and
the second tile's carry is injected through the matching db column, which
keeps the recurrence exact). Valid only when A is constant across channels
(standard Mamba init) - verified on the host per call, with an unmerged
fallback build. db=du*B mostly on GPSIMD (1/6 on DVE to balance), gg=h*C
on DVE, y accumulation over n via identity matmuls into PSUM seeded by a
diag(D) matmul -> silu(z) gate -> out_proj partial (f16).

The two chunks are software-pipelined at emission level: each chunk's head
(LN/in_proj/conv/x_proj/dt) is spliced into the previous chunk's scan loop
at iteration indices chosen to respect every tile-buffer WAR hazard
(xpad/xc generation cycling, per-i dl/du single buffering, PSUM bank
rotation), and the previous chunk's tail (gate + out_proj) is spliced into
the next scan. DRAM bounce buffers (B/C rows, z spill) ping-pong per chunk
parity so per-tensor DRAM dependency tracking cannot serialize chunks.
Host only does slicing / transposes / flips (layout, no math).
"""

import sys
import numpy as np

sys.path.insert(0, "/opt/trn_rl_repo")

B, L, D, E, N, KC, R = 4, 2048, 512, 1024, 16, 4, 32
EPS = 1e-5
ET = E // 128       # 8 e-tiles
DT = D // 128       # 4 d-tiles
TL = 1024           # L chunk size
BLK = 2             # scan i-block size (ypss PSUM tiles alive at once)
DTG = 2             # dt_proj exp/ln batching group (i's per table-set switch)
NL = L // TL        # chunks
NSUB = TL // 512    # 512-wide matmul subchunks per chunk

_cache = {}


def _build_stage_a(reps=1, merged=True):
    import concourse.tile as tile
    from concourse import bacc, mybir
    from concourse.alu_op_type import AluOpType as op
    from contextlib import ExitStack

    dt = mybir.dt
    f32, f16 = dt.float32, dt.float16
    AF = mybir.ActivationFunctionType

    nc = bacc.Bacc("TRN2", target_bir_lowering=False, debug=False, num_devices=8)

    # ---- DRAM I/O (per-core values supplied via in_maps) ----
    hsT = nc.dram_tensor("hsT", [D, L], f16, kind="ExternalInput").ap()
    w_inT = nc.dram_tensor("w_inT", [D, 2 * E], f16, kind="ExternalInput").ap()
    out_wT = nc.dram_tensor("out_wT", [E, D], f16, kind="ExternalInput").ap()
    xp_wT = nc.dram_tensor("xp_wT", [E, R + 2 * N], f16, kind="ExternalInput").ap()
    dtp_wT = nc.dram_tensor("dtp_wT", [R, E], f16, kind="ExternalInput").ap()
    # packed per-partition columns: [conv_w(4) per tile | conv_b | dt_b | D | norm cols]
    convw = nc.dram_tensor("convw", [128, ET * KC], f32, kind="ExternalInput").ap()
    convb = nc.dram_tensor("convb", [128, ET], f32, kind="ExternalInput").ap()
    dtb = nc.dram_tensor("dtb", [128, ET], f32, kind="ExternalInput").ap()
    dvec = nc.dram_tensor("dvec", [128, ET], f32, kind="ExternalInput").ap()
    avals = nc.dram_tensor("avals", [128, ET * N], f32, kind="ExternalInput").ap()
    nw = nc.dram_tensor("nw", [128, DT], f32, kind="ExternalInput").ap()
    nb = nc.dram_tensor("nb", [128, DT], f32, kind="ExternalInput").ap()
    y_part = nc.dram_tensor("y_part", [D, L], f16, kind="ExternalOutput").ap()
    # ping-pong bounce buffers (parity per chunk) so chunk c+1's writes don't
    # serialize against chunk c's reads via per-tensor DRAM dep tracking
    bcd = [nc.dram_tensor(f"bcd{p}", [2 * N, TL], f16).ap() for p in range(2)]
    zdram = [nc.dram_tensor(f"zdram{p}", [E, TL], f16).ap() for p in range(2)]

    with tile.TileContext(nc) as tc:
        with ExitStack() as ctx:
            P = 128

            def pool(name, bufs):
                return ctx.enter_context(tc.tile_pool(name=name, bufs=bufs))

            pers = pool("pers", 1)
            ps_pool = ctx.enter_context(tc.tile_pool(name="ps", bufs=3, space="PSUM"))
            ps_aux = ctx.enter_context(tc.tile_pool(name="psaux", bufs=1, space="PSUM"))
            ps_y = ctx.enter_context(tc.tile_pool(name="psy", bufs=2, space="PSUM"))

            # ---- persistent weight tiles ----
            w_in = [pers.tile([P, 2 * E], f16, tag=f"win{k}", name=f"win{k}") for k in range(DT)]
            for k in range(DT):
                nc.sync.dma_start(w_in[k][:], w_inT[128 * k:128 * (k + 1), :])
            out_w = [pers.tile([P, D], f16, tag=f"ow{i}", name=f"ow{i}") for i in range(ET)]
            def load_out_w():
                for i in range(ET):
                    nc.sync.dma_start(out_w[i][:], out_wT[128 * i:128 * (i + 1), :])
            xp_w = [pers.tile([P, R + 2 * N], f16, tag=f"xpw{i}", name=f"xpw{i}") for i in range(ET)]
            for i in range(ET):
                nc.sync.dma_start(xp_w[i][:], xp_wT[128 * i:128 * (i + 1), :])
            dtp_w = pers.tile([R, E], f16, tag="dtpw", name="dtpw")
            nc.sync.dma_start(dtp_w[:], dtp_wT[:])
            cw = pers.tile([P, ET * KC], f32, tag="cw", name="cw")
            nc.sync.dma_start(cw[:], convw[:])
            cb = pers.tile([P, ET], f32, tag="cb", name="cb")
            nc.sync.dma_start(cb[:], convb[:])
            dtbt = pers.tile([P, ET], f32, tag="dtb", name="dtb")
            nc.sync.dma_start(dtbt[:], dtb[:])
            dvt = pers.tile([P, ET], f32, tag="dv", name="dv")
            nc.sync.dma_start(dvt[:], dvec[:])
            # A = -exp(A_log) precomputed on host
            At = pers.tile([P, ET * N], f32, tag="A", name="A")
            nc.sync.dma_start(At[:], avals[:])
            nwt = pers.tile([P, DT], f32, tag="nw", name="nw")
            nc.sync.dma_start(nwt[:], nw[:])
            nbt = pers.tile([P, DT], f32, tag="nb", name="nb")
            nc.sync.dma_start(nbt[:], nb[:])

            ones = pers.tile([P, 1], f16, tag="ones", name="ones")
            nc.vector.memset(ones[:], 1.0)
            from concourse import masks
            ident = pers.tile([P, P], f16, tag="ident", name="ident")
            masks.make_identity(nc, ident[:])
            epst = pers.tile([P, 1], f32, tag="epst", name="epst")
            nc.vector.memset(epst[:], EPS)

            # depthwise-conv taps as diagonal matrices (PE lhsT)
            dgw = [[pers.tile([P, P], f16, tag=f"dg{i}_{k}", name=f"dg{i}_{k}")
                    for k in range(KC)] for i in range(ET)]
            for i in range(ET):
                for k in range(KC):
                    nc.vector.tensor_scalar_mul(dgw[i][k][:], ident[:],
                                                cw[:, KC * i + k:KC * i + k + 1])

            # D-vector as diagonal matrices for the y-seed matmul
            ddv = [pers.tile([P, P], f16, tag=f"ddv{i}", name=f"ddv{i}") for i in range(ET)]
            for i in range(ET):
                nc.vector.tensor_scalar_mul(ddv[i][:], ident[:], dvt[:, i:i + 1])

            # scan carry state h[:, (i,n)] and conv tails
            carry = [pers.tile([P, N], f16, tag=f"carry{i}", name=f"carry{i}") for i in range(ET)]
            xtail = [pers.tile([P, 3], f16, tag=f"xtail{i}", name=f"xtail{i}") for i in range(ET)]
            for i in range(ET):
                nc.vector.memset(xtail[i][:], 0.0)

            # ---- streaming pools ----
            hst_p = pool("hst", 1)     # hsT chunk tiles (f16)
            sq_p = pool("sq", 1)       # squared sub-tiles (f16, transient)
            srow_p = pool("srow", 1)   # stat rows (1, TL) f16
            rep_p = pool("rep", 1)     # broadcast stat rows (128, TL) f16
            hn_p = pool("hn", 1)       # normalized hs (f16), DT tags
            xpad_p = pool("xpad", 3)   # conv input [3 | TL] f16, cycled
            xc_p = pool("xc", 1)       # conv output f16, ET tags
            z_p = pool("z", 2)         # z spill bounce (small)
            xdbl_p = pool("xdbl", 1)   # (64, TL) f16
            esp_p = pool("esp", 1)     # exp(dt_raw) staging f32, DTG tags
            dl_p = pool("dl", 1)       # delta f16, ET tags (fully resident)
            du_p = pool("du", 1)       # delta*u f16, ET tags (fully resident)
            y_p = pool("y", 1)         # y gated f16, ET tags
            bc_p = pool("bc", 2)       # B/C broadcast planes f16
            tr_p = pool("tr", 2)       # scan transients f16
            ov_p = pool("ov", 1)       # out_proj result f16
            tmp_p = pool("tmp", 1)     # misc small

            import itertools

            chunks = [(rep, c) for rep in range(reps) for c in range(NL)]
            NC = len(chunks)
            states = [dict() for _ in range(NC)]

            def emit_LN(ci):
                st = states[ci]
                lo = chunks[ci][1] * TL
                hst = []
                for k in range(DT):
                    t = hst_p.tile([P, TL], f16, tag=f"hst{k}", name=f"hst{k}")
                    nc.sync.dma_start(t[:], hsT[128 * k:128 * (k + 1), lo:lo + TL])
                    hst.append(t)
                mu = srow_p.tile([1, TL], f16, tag="mu", name="mu")
                msq = srow_p.tile([1, TL], f16, tag="msq", name="msq")
                for s in range(NSUB):
                    sl = slice(512 * s, 512 * (s + 1))
                    st_ps = ps_aux.tile([33, 512], f32, tag="aux", name="stps",
                                        padded_shape=[128, 512])
                    mu_ps, sq_ps = st_ps[0:1, :], st_ps[32:33, :]
                    for k in range(DT):
                        sqt = sq_p.tile([P, 512], f16, tag="sqt", name="sqt")
                        nc.vector.tensor_tensor(sqt[:], hst[k][:, sl], hst[k][:, sl], op=op.mult)
                        nc.tensor.matmul(mu_ps, ones[:], hst[k][:, sl], skip_group_check=True,
                                         start=(k == 0), stop=(k == DT - 1))
                        nc.tensor.matmul(sq_ps, ones[:], sqt[:], skip_group_check=True,
                                         start=(k == 0), stop=(k == DT - 1))
                    nc.scalar.activation(mu[:, sl], mu_ps, AF.Copy, scale=1.0 / D)
                    nc.scalar.activation(msq[:, sl], sq_ps, AF.Copy, scale=1.0 / D)
                mu2 = srow_p.tile([1, TL], f16, tag="mu2", name="mu2")
                nc.vector.tensor_tensor(mu2[:], mu[:], mu[:], op=op.mult)
                nc.vector.tensor_sub(msq[:], msq[:], mu2[:])
                nc.scalar.activation(msq[:], msq[:], AF.Sqrt, bias=epst[0:1, :])
                with nc.allow_low_precision(reason="rsqrt row in f16; |rs|~1"):
                    nc.vector.reciprocal(mu2[:], msq[:])
                murep = rep_p.tile([P, TL], f16, tag="murep", name="murep")
                nc.gpsimd.partition_broadcast(murep[:], mu[:])
                rsrep = rep_p.tile([P, TL], f16, tag="rsrep", name="rsrep")
                nc.gpsimd.partition_broadcast(rsrep[:], mu2[:])
                hn = []
                for k in range(DT):
                    rsw = tmp_p.tile([P, TL], f16, tag="rsw", name="rsw")
                    nc.vector.tensor_scalar_mul(rsw[:], rsrep[:], nwt[:, k:k + 1])
                    bias2 = tmp_p.tile([P, TL], f16, tag="bias2", name="bias2")
                    nc.vector.tensor_tensor(bias2[:], murep[:], rsw[:], op=op.mult)
                    nc.vector.tensor_scalar(bias2[:], bias2[:], -1.0, nbt[:, k:k + 1],
                                            op0=op.mult, op1=op.add)
                    ht = hn_p.tile([P, TL], f16, tag=f"hn{k}", name=f"hn{k}")
                    nc.vector.tensor_tensor(ht[:], hst[k][:], rsw[:], op=op.mult)
                    nc.vector.tensor_add(ht[:], ht[:], bias2[:])
                    hn.append(ht)
                st["hn"] = hn
                st["xpads"] = []
                st["xcs"] = []
                st["xcps"] = []
                st["dls"] = []
                st["dus"] = []
                st["dlp"] = []
                st["dup"] = []
                st["esps"] = {}

            def emit_inproj_m(ci, m):
                st = states[ci]
                hn = st["hn"]
                xp = xpad_p.tile([P, TL + 3], f16, tag="xpad", name="xpad")
                nc.scalar.copy(xp[:, 0:3], xtail[m][:])
                st["xpads"].append(xp)
                for s in range(NSUB):
                    sl = slice(512 * s, 512 * (s + 1))
                    ps = ps_pool.tile([P, 512], f32, tag="mm", name="mm")
                    for k in range(DT):
                        nc.tensor.matmul(ps[:],
                                         w_in[k][:, 128 * m:128 * (m + 1)],
                                         hn[k][:, sl],
                                         start=(k == 0), stop=(k == DT - 1))
                    nc.scalar.copy(xp[:, 3 + 512 * s:3 + 512 * (s + 1)], ps[:])

            def emit_z(ci, mz):
                st = states[ci]
                hn = st["hn"]
                lo_par = ci % 2
                m = ET + mz
                zt = z_p.tile([P, TL], f16, tag="zsp", name="zsp")
                for s in range(NSUB):
                    sl = slice(512 * s, 512 * (s + 1))
                    ps = ps_pool.tile([P, 512], f32, tag="mm", name="mm")
                    for k in range(DT):
                        nc.tensor.matmul(ps[:],
                                         w_in[k][:, 128 * m:128 * (m + 1)],
                                         hn[k][:, sl],
                                         start=(k == 0), stop=(k == DT - 1))
                    nc.scalar.copy(zt[:, sl], ps[:])
                nc.sync.dma_start(zdram[lo_par][128 * mz:128 * (mz + 1), :], zt[:])

            def emit_conv(ci, i):
                st = states[ci]
                xp = st["xpads"][i]
                nc.scalar.copy(xtail[i][:], xp[:, TL:TL + 3])
                if merged:
                    if i % 2 == 0:
                        xcp = xc_p.tile([P, 2 * TL], f16, tag=f"xcp{i // 2}",
                                        name=f"xcp{i // 2}",
                                        bufs=2 if i >= 6 else 1)
                        st["xcps"].append(xcp)
                    xcp = st["xcps"][i // 2]
                    xct = xcp[:, (i % 2) * TL:(i % 2 + 1) * TL]
                else:
                    xct = xc_p.tile([P, TL], f16, tag=f"xc{i}", name=f"xc{i}",
                                    bufs=2 if i >= 6 else 1)
                for s in range(NSUB):
                    sl = slice(512 * s, 512 * (s + 1))
                    psc = ps_pool.tile([P, 512], f32, tag="mm", name="mmc")
                    for k in range(KC):
                        nc.tensor.matmul(psc[:], dgw[i][k][:],
                                         xp[:, 512 * s + k:512 * s + k + 512],
                                         start=(k == 0), stop=(k == KC - 1))
                    nc.scalar.activation(xct[:, sl], psc[:], AF.Silu, bias=cb[:, i:i + 1])
                st["xcs"].append(xct)

            def emit_xproj(ci):
                st = states[ci]
                xcs = st["xcs"]
                xdbl = xdbl_p.tile([R, TL], f16, tag="xdbl", name="xdbl")
                bcs = xdbl_p.tile([2 * N, TL], f16, tag="bcs", name="bcs", bufs=1)
                for s in range(NSUB):
                    sl = slice(512 * s, 512 * (s + 1))
                    ps = ps_aux.tile([R + 2 * N, 512], f32, tag="aux", name="xdblps",
                                     padded_shape=[128, 512])
                    for i in range(ET):
                        nc.tensor.matmul(ps[:], xp_w[i][:], xcs[i][:, sl],
                                         start=(i == 0), stop=(i == ET - 1))
                    nc.scalar.copy(xdbl[:, sl], ps[0:R, :])
                    nc.scalar.copy(bcs[:, sl], ps[R:R + 2 * N, :])
                nc.sync.dma_start(bcd[ci % 2][:, :], bcs[:])
                st["xdbl"] = xdbl

            def emit_dt_exps(ci, g):
                st = states[ci]
                xdbl = st["xdbl"]
                if merged:
                    if g % 2 == 0:
                        st.setdefault("quad_esp", {})[g // 2] = esp_p.tile(
                            [P, 4 * TL], f16, tag="espq", name=f"espq{g // 2}")
                    st["esps"][g] = st["quad_esp"][g // 2][:, (g % 2) * 2 * TL:(g % 2 + 1) * 2 * TL]
                for ii in range(DTG):
                    i = g * DTG + ii
                    if merged:
                        esp = st["esps"][g][:, ii * TL:(ii + 1) * TL]
                    else:
                        esp = esp_p.tile([P, TL], f16, tag=f"esp{ii}", name=f"esp{i}")
                        st["esps"][i] = esp
                    for s in range(NSUB):
                        sl = slice(512 * s, 512 * (s + 1))
                        ps = ps_pool.tile([P, 512], f32, tag="mm", name="mm")
                        nc.tensor.matmul(ps[:], dtp_w[:, 128 * i:128 * (i + 1)],
                                         xdbl[0:R, sl], start=True, stop=True)
                        nc.scalar.activation(esp[:, sl], ps[:], AF.Exp,
                                             bias=dtbt[:, i:i + 1])

            def emit_dt_ln_du(ci, i):
                # merged: i is a QUAD index q (4 e-tiles); else a single e-tile
                st = states[ci]
                if merged:
                    q = i
                    dlq = dl_p.tile([P, 4 * TL], f16, tag=f"dlq{q}", name=f"dlq{q}")
                    nc.scalar.activation(dlq[:], st["quad_esp"][q][:], AF.Ln, bias=1.0)
                    for gh in range(2):
                        g = 2 * q + gh
                        dlp = dlq[:, gh * 2 * TL:(gh + 1) * 2 * TL]
                        st["dlp"].append(dlp)
                        st["dls"].extend([dlp[:, 0:TL], dlp[:, TL:2 * TL]])
                        dup = du_p.tile([P, 2 * TL], f16, tag=f"dup{g}", name=f"dup{g}")
                        nc.vector.tensor_tensor(dup[:], dlp[:], st["xcps"][g][:], op=op.mult)
                        st["dup"].append(dup)
                        st["dus"].extend([dup[:, 0:TL], dup[:, TL:2 * TL]])
                else:
                    dl = dl_p.tile([P, TL], f16, tag=f"dl{i}", name=f"dl{i}")
                    nc.scalar.activation(dl[:], st["esps"][i][:], AF.Ln, bias=1.0)
                    st["dls"].append(dl)
                    du = du_p.tile([P, TL], f16, tag=f"du{i}", name=f"du{i}")
                    nc.vector.tensor_tensor(du[:], dl[:], st["xcs"][i][:], op=op.mult)
                    st["dus"].append(du)

            def emit_gate(ci, i):
                st = states[ci]
                zr = z_p.tile([P, TL], f16, tag="zr", name="zr")
                nc.sync.dma_start(zr[:], zdram[ci % 2][128 * i:128 * (i + 1), :])
                zst = tmp_p.tile([P, TL], f16, tag="zs", name="zs")
                nc.scalar.activation(zst[:], zr[:], AF.Silu)
                nc.vector.tensor_tensor(st["yfin"][i][:], st["yfin"][i][:], zst[:], op=op.mult)

            def emit_outproj(ci, m):
                st = states[ci]
                lo = chunks[ci][1] * TL
                ygs = st["yfin"]
                ov = ov_p.tile([P, TL], f16, tag="ov", name="ov")
                for s in range(NSUB):
                    sl = slice(512 * s, 512 * (s + 1))
                    ps = ps_pool.tile([P, 512], f32, tag="mm", name="mm")
                    for i in range(ET):
                        nc.tensor.matmul(ps[:],
                                         out_w[i][:, 128 * m:128 * (m + 1)],
                                         ygs[i][:, sl],
                                         start=(i == 0), stop=(i == ET - 1))
                    nc.scalar.copy(ov[:, sl], ps[:])
                nc.sync.dma_start(y_part[128 * m:128 * (m + 1), lo:lo + TL], ov[:])

            def head_schedule(ci, shift=0):
                """Pacing of chunk ci's head into chunk ci-1's scan loop.
                Keys are scan iteration index it = ib*16 + n (0..63)."""
                S = {}
                def sh(d):
                    return {min(k + shift, 63): v for k, v in d.items()}
                S[1] = [lambda: emit_LN(ci)]
                if ci == 1:
                    S[1] = S[1] + [load_out_w]
                S[2] = [lambda: emit_inproj_m(ci, 0)]
                S[3] = [lambda: emit_inproj_m(ci, 1)]
                S[4] = [lambda: emit_inproj_m(ci, 2)]
                S[6] = [lambda: emit_conv(ci, 0), lambda: emit_conv(ci, 1)]
                S[7] = [lambda: emit_inproj_m(ci, 3)]
                S[8] = [lambda: emit_inproj_m(ci, 4)]
                for j in range(4):
                    S[9 + j] = [lambda j=j: emit_z(ci, j)]
                S[17] = [lambda: emit_conv(ci, 2), lambda: emit_conv(ci, 3)]
                S[18] = [lambda: emit_inproj_m(ci, 5)]
                S[19] = [lambda: emit_inproj_m(ci, 6)]
                for j in range(4):
                    S[20 + j] = [lambda j=j: emit_z(ci, 4 + j)]
                S[33] = [lambda: emit_conv(ci, 4), lambda: emit_conv(ci, 5)]
                S[34] = [lambda: emit_inproj_m(ci, 7)]
                S[35] = [lambda: emit_conv(ci, 6), lambda: emit_conv(ci, 7)]
                S[36] = [lambda: emit_xproj(ci)]
                S[37] = [lambda: emit_dt_exps(ci, 0)]
                S[40] = [lambda: emit_dt_exps(ci, 1)]
                if merged:
                    S[42] = [lambda: emit_dt_ln_du(ci, 0)]
                    S[43] = [lambda: emit_dt_exps(ci, 2)]
                    S[50] = [lambda: emit_dt_exps(ci, 3)]
                    leftover = [lambda: emit_dt_ln_du(ci, 1)]
                else:
                    S[43] = [lambda: emit_dt_exps(ci, 2)]
                    S[44] = [lambda: emit_dt_exps(ci, 3)]
                    S[39] = [lambda: emit_dt_ln_du(ci, 0), lambda: emit_dt_ln_du(ci, 1)]
                    S[42] = [lambda: emit_dt_ln_du(ci, 2), lambda: emit_dt_ln_du(ci, 3)]
                    S[49] = [lambda: emit_dt_ln_du(ci, 4), lambda: emit_dt_ln_du(ci, 5)]
                    leftover = [lambda: emit_dt_ln_du(ci, 6), lambda: emit_dt_ln_du(ci, 7)]
                if shift:
                    shifted = {}
                    for k, v in S.items():
                        shifted.setdefault(min(k + shift, 61), []).extend(v)
                    S = shifted
                return S, leftover

            def tail_schedule(ci):
                """Pacing of chunk ci's tail (gate + out_proj) into scan(ci+1)."""
                S = {}
                S[0] = [lambda i=i: emit_gate(ci, i) for i in range(ET)]
                for m in range(DT):
                    S.setdefault(8 + 2 * m, []).append(lambda m=m: emit_outproj(ci, m))
                return S

            def emit_scan(ci, sched, inline_gate=False):
                st = states[ci]
                c = chunks[ci][1]
                dls, dus, xcs = st["dls"], st["dus"], st["xcs"]
                ypss, yfin = {}, {}
                st["yfin"] = yfin
                for ib, n in itertools.product(range(ET // BLK), range(N)):
                    it = ib * N + n
                    if merged:
                        W = 2 * TL
                        bp = bc_p.tile([P, TL], f16, tag="bp", name="bp")
                        nc.sync.dma_start(bp[:], bcd[ci % 2][n:n + 1, :].to_broadcast((P, TL)))
                        cp = bc_p.tile([P, TL], f16, tag="cp", name="cp")
                        nc.sync.dma_start(cp[:], bcd[ci % 2][N + n:N + n + 1, :].to_broadcast((P, TL)))
                        i0 = BLK * ib
                        if n == 0:
                            for i in range(i0, i0 + BLK):
                                yp = ps_y.tile([P, TL], f32, tag="yps", name="yps")
                                ypss[i] = yp
                                for sb in range(NSUB):
                                    sl = slice(512 * sb, 512 * (sb + 1))
                                    nc.tensor.matmul(yp[:, sl], ddv[i][:], xcs[i][:, sl],
                                                     start=True, stop=False)
                        da = tr_p.tile([P, W], f16, tag="da", name="da", bufs=2)
                        nc.scalar.activation(da[:], st["dlp"][ib][:], AF.Exp,
                                             scale=At[:, N * i0 + n:N * i0 + n + 1])
                        db = tr_p.tile([P, W], f16, tag="db", name="db", bufs=2)
                        for h in range(2):
                            hsl = slice(h * TL, (h + 1) * TL)
                            if ((i0 + h) * N + n) % 6 == 0:
                                nc.vector.tensor_tensor(db[:, hsl], st["dup"][ib][:, hsl],
                                                        bp[:], op=op.mult)
                            else:
                                nc.gpsimd.tensor_tensor(db[:, hsl], st["dup"][ib][:, hsl],
                                                        bp[:], op=op.mult)
                        if c > 0:
                            # inject e-tile i0+1's carry through the boundary column
                            nc.vector.scalar_tensor_tensor(
                                db[:, TL:TL + 1], da[:, TL:TL + 1],
                                carry[i0 + 1][:, n:n + 1], db[:, TL:TL + 1],
                                op0=op.mult, op1=op.add)
                        nc.vector.memset(da[:, TL:TL + 1], 0.0)
                        hh = tr_p.tile([P, W], f16, tag="hh", name="hh", bufs=2)
                        init = 0.0 if c == 0 else carry[i0][:, n:n + 1]
                        nc.vector.tensor_tensor_scan(hh[:], da[:], db[:], init,
                                                     op0=op.mult, op1=op.add)
                        gg = tr_p.tile([P, W], f16, tag="gg", name="gg", bufs=2)
                        for h in range(2):
                            hsl = slice(h * TL, (h + 1) * TL)
                            nc.vector.tensor_tensor(gg[:, hsl], hh[:, hsl], cp[:], op=op.mult)
                        if c < NL - 1:
                            for h in range(2):
                                nc.vector.tensor_copy(carry[i0 + h][:, n:n + 1],
                                                      hh[:, (h + 1) * TL - 1:(h + 1) * TL])
                        for i in range(i0, i0 + BLK):
                            off = (i - i0) * TL
                            for sb in range(NSUB):
                                sl = slice(off + 512 * sb, off + 512 * (sb + 1))
                                osl = slice(512 * sb, 512 * (sb + 1))
                                nc.tensor.matmul(ypss[i][:, osl], ident[:], gg[:, sl],
                                                 start=False, stop=(n == N - 1))
                            if n == N - 1:
                                yt = y_p.tile([P, TL], f16, tag=f"y{i}", name=f"y{i}")
                                nc.scalar.copy(yt[:], ypss[i][:])
                                yfin[i] = yt
                                if inline_gate:
                                    emit_gate(ci, i)
                        for thunk in sched.get(it, ()):
                            thunk()
                        continue
                    bp = bc_p.tile([P, TL], f16, tag="bp", name="bp")
                    nc.sync.dma_start(bp[:], bcd[ci % 2][n:n + 1, :].to_broadcast((P, TL)))
                    cp = bc_p.tile([P, TL], f16, tag="cp", name="cp")
                    nc.sync.dma_start(cp[:], bcd[ci % 2][N + n:N + n + 1, :].to_broadcast((P, TL)))
                    for i in range(BLK * ib, BLK * (ib + 1)):
                        if n == 0:
                            yp = ps_y.tile([P, TL], f32, tag="yps", name="yps")
                            ypss[i] = yp
                            for sb in range(NSUB):
                                sl = slice(512 * sb, 512 * (sb + 1))
                                nc.tensor.matmul(yp[:, sl], ddv[i][:], xcs[i][:, sl],
                                                 start=True, stop=False)
                        da = tr_p.tile([P, TL], f16, tag="da", name="da", bufs=3)
                        nc.scalar.activation(da[:], dls[i][:], AF.Exp,
                                             scale=At[:, N * i + n:N * i + n + 1])
                        db = tr_p.tile([P, TL], f16, tag="db", name="db", bufs=3)
                        if (i * N + n) % 7 == 0:
                            nc.vector.tensor_tensor(db[:], dus[i][:], bp[:], op=op.mult)
                        else:
                            nc.gpsimd.tensor_tensor(db[:], dus[i][:], bp[:], op=op.mult)
                        hh = tr_p.tile([P, TL], f16, tag="hh", name="hh", bufs=3)
                        init = 0.0 if c == 0 else carry[i][:, n:n + 1]
                        nc.vector.tensor_tensor_scan(hh[:], da[:], db[:], init,
                                                     op0=op.mult, op1=op.add)
                        gg = tr_p.tile([P, TL], f16, tag="gg", name="gg", bufs=3)
                        nc.vector.tensor_tensor(gg[:], hh[:], cp[:], op=op.mult)
                        if c < NL - 1:
                            nc.vector.tensor_copy(carry[i][:, n:n + 1], hh[:, TL - 1:TL])
                        for sb in range(NSUB):
                            sl = slice(512 * sb, 512 * (sb + 1))
                            nc.tensor.matmul(ypss[i][:, sl], ident[:], gg[:, sl],
                                             start=False, stop=(n == N - 1))
                        if n == N - 1:
                            yt = y_p.tile([P, TL], f16, tag=f"y{i}", name=f"y{i}")
                            nc.scalar.copy(yt[:], ypss[i][:])
                            yfin[i] = yt
                            if inline_gate:
                                emit_gate(ci, i)
                    for thunk in sched.get(it, ()):
                        thunk()

            # ---- software-pipelined emission over all chunks ----
            # chunk 0: emit only the critical head path before scan(0); the
            # z half and the late dt groups are paced into scan(0) itself.
            emit_LN(0)
            for m in (0, 1, 2):
                emit_inproj_m(0, m)
            emit_conv(0, 0); emit_conv(0, 1)
            for m in (3, 4):
                emit_inproj_m(0, m)
            emit_conv(0, 2); emit_conv(0, 3)
            for m in (5, 6):
                emit_inproj_m(0, m)
            emit_conv(0, 4); emit_conv(0, 5)
            emit_inproj_m(0, 7)
            emit_conv(0, 6); emit_conv(0, 7)
            emit_xproj(0)
            emit_dt_exps(0, 0)
            if merged:
                emit_dt_exps(0, 1)
                emit_dt_ln_du(0, 0)
                head0_rest = {1: [lambda: emit_dt_exps(0, 2)],
                              4: [lambda: emit_dt_exps(0, 3)],
                              5: [lambda: emit_dt_ln_du(0, 1)]}
            else:
                emit_dt_ln_du(0, 0); emit_dt_ln_du(0, 1)
                head0_rest = {1: [lambda: emit_dt_exps(0, 1)],
                              2: [lambda: emit_dt_ln_du(0, 2), lambda: emit_dt_ln_du(0, 3)],
                              4: [lambda: emit_dt_exps(0, 2)],
                              5: [lambda: emit_dt_ln_du(0, 4), lambda: emit_dt_ln_du(0, 5)],
                              7: [lambda: emit_dt_exps(0, 3)],
                              8: [lambda: emit_dt_ln_du(0, 6), lambda: emit_dt_ln_du(0, 7)]}
            for j in range(8):
                head0_rest.setdefault(3 + j, []).append(lambda j=j: emit_z(0, j))
            for ci in range(NC):
                sched = {}
                if ci == 0:
                    for it, ts in head0_rest.items():
                        sched.setdefault(it, []).extend(ts)
                if ci > 0:
                    for it, ts in tail_schedule(ci - 1).items():
                        sched.setdefault(it, []).extend(ts)
                leftover = []
                if ci + 1 < NC:
                    hS, leftover = head_schedule(ci + 1, shift=12 if ci == 0 else 0)
                    for it, ts in hS.items():
                        sched.setdefault(it, []).extend(ts)
                emit_scan(ci, sched, inline_gate=(ci == NC - 1))
                for thunk in leftover:
                    thunk()
            for m in range(DT):
                emit_outproj(NC - 1, m)

    nc.compile()
    return nc


def _build_stage_b(reps=1):
    import concourse.tile as tile
    from concourse import bacc, mybir
    from concourse.alu_op_type import AluOpType as op
    from contextlib import ExitStack

    dt = mybir.dt
    f32, f16 = dt.float32, dt.float16
    AF = mybir.ActivationFunctionType
    LH = L // 2  # 1024 rows per core

    nc = bacc.Bacc("TRN2", target_bir_lowering=False, debug=False, num_devices=8)
    yf = nc.dram_tensor("yf", [LH, D], f16, kind="ExternalInput").ap()
    yr = nc.dram_tensor("yr", [LH, D], f16, kind="ExternalInput").ap()
    res = nc.dram_tensor("res", [LH, D], f16, kind="ExternalInput").ap()
    nfw = nc.dram_tensor("nfw", [1, D], f32, kind="ExternalInput").ap()
    out = nc.dram_tensor("out", [LH, D], f32, kind="ExternalOutput").ap()

    with tile.TileContext(nc) as tc:
        with ExitStack() as ctx:
            P = 128
            pers = ctx.enter_context(tc.tile_pool(name="pers", bufs=1))
            io_p = ctx.enter_context(tc.tile_pool(name="io", bufs=8))
            tmp_p = ctx.enter_context(tc.tile_pool(name="tmp", bufs=8))

            epst = pers.tile([128, 1], f32, tag="epst", name="epst")
            nc.vector.memset(epst[:], EPS)
            nfwt = pers.tile([1, D], f32, tag="nfw", name="nfw")
            nc.sync.dma_start(nfwt[:], nfw[:])
            nfr = pers.tile([P, D], f32, tag="nfr", name="nfr")
            nc.gpsimd.partition_broadcast(nfr[:], nfwt[:])

            import itertools
            for rep, t in itertools.product(range(reps), range(LH // P)):
                rows = slice(P * t, P * (t + 1))
                tf = io_p.tile([P, D], f16, tag="tf", name="tf")
                nc.sync.dma_start(tf[:], yf[rows, :])
                tr = io_p.tile([P, D], f16, tag="tr", name="tr")
                nc.sync.dma_start(tr[:], yr[rows, :])
                tres = io_p.tile([P, D], f16, tag="tres", name="tres")
                nc.sync.dma_start(tres[:], res[rows, :])
                s = tmp_p.tile([P, D], f32, tag="s", name="s")
                nc.vector.tensor_add(s[:], tf[:], tr[:])
                nc.vector.tensor_add(s[:], s[:], tres[:])
                sq = tmp_p.tile([P, D], f32, tag="sq", name="sq")
                ssum = tmp_p.tile([P, 1], f32, tag="ssum", name="ssum")
                nc.scalar.activation(sq[:], s[:], AF.Square, accum_out=ssum[:])
                lnm = tmp_p.tile([P, 1], f32, tag="lnm", name="lnm")
                nc.scalar.activation(lnm[:], ssum[:], AF.Sqrt, bias=epst[:], scale=1.0 / D)
                rinv = tmp_p.tile([P, 1], f32, tag="rinv", name="rinv")
                nc.vector.reciprocal(rinv[:], lnm[:])
                o = tmp_p.tile([P, D], f32, tag="o", name="o")
                nc.vector.scalar_tensor_tensor(o[:], s[:], rinv[:], nfr[:],
                                               op0=op.mult, op1=op.mult)
                nc.sync.dma_start(out[rows, :], o[:])

    nc.compile()
    return nc


class _Runner:
    """Compile a Bass program once into a sharded PJRT callable for 8 cores."""

    def __init__(self, nc, n_cores=8):
        import jax
        import jax.numpy as jnp
        from jax.sharding import Mesh, PartitionSpec
        from jax.experimental.shard_map import shard_map
        from concourse import bass2jax, mybir

        bass2jax.install_neuronx_cc_hook()
        self.n_cores = n_cores
        in_names, out_names, out_avals, zero_outs = [], [], [], []
        partition_name = nc.partition_id_tensor.name if nc.partition_id_tensor else None
        for alloc in nc.m.functions[0].allocations:
            if not isinstance(alloc, mybir.MemoryLocationSet):
                continue
            name = alloc.memorylocations[0].name
            if alloc.kind == "ExternalInput":
                if name != partition_name:
                    in_names.append(name)
            elif alloc.kind == "ExternalOutput":
                shape = tuple(alloc.tensor_shape)
                dtype = mybir.dt.np(alloc.dtype)
                out_names.append(name)
                out_avals.append(jax.core.ShapedArray(shape, dtype))
                zero_outs.append(np.zeros((n_cores * shape[0],) + shape[1:], dtype))
        self.in_names, self.out_names, self.out_avals = in_names, out_names, out_avals
        n_params, n_outs = len(in_names), len(out_names)
        all_names = list(in_names) + list(out_names)
        if partition_name is not None:
            all_names.append(partition_name)

        def _body(*args):
            operands = list(args)
            if partition_name is not None:
                operands.append(bass2jax.partition_id_tensor())
            outs = bass2jax._bass_exec_p.bind(
                *operands,
                out_avals=tuple(out_avals),
                in_names=tuple(all_names),
                out_names=tuple(out_names),
                lowering_input_output_aliases=(),
                sim_require_finite=True,
                sim_require_nnan=True,
                nc=nc,
            )
            return tuple(outs)

        devices = jax.devices()[:n_cores]
        mesh = Mesh(np.asarray(devices), ("core",))
        in_specs = (PartitionSpec("core"),) * (n_params + n_outs)
        out_specs = (PartitionSpec("core"),) * n_outs
        self.fn = jax.jit(
            shard_map(_body, mesh=mesh, in_specs=in_specs,
                      out_specs=out_specs, check_rep=False),
            keep_unused=True)
        self.mesh = mesh
        self._zero_dev = [jax.device_put(z) for z in zero_outs]

    def prep(self, in_maps):
        import jax
        assert len(in_maps) == self.n_cores
        concat = [np.concatenate([np.asarray(m[n]) for m in in_maps], axis=0)
                  for n in self.in_names]
        return [jax.device_put(a) for a in concat] + self._zero_dev

    def run_dev(self, dev_args):
        return self.fn(*dev_args)

    def __call__(self, in_maps):
        import jax
        out_arrs = self.fn(*self.prep(in_maps))
        out_arrs = [np.asarray(a) for a in out_arrs]
        res = []
        for c in range(self.n_cores):
            d = {}
            for i, name in enumerate(self.out_names):
                shape = self.out_avals[i].shape
                d[name] = out_arrs[i].reshape((self.n_cores,) + shape)[c]
            res.append(d)
        return res


def _programs(merged=True):
    key = ("a", merged)
    if key not in _cache:
        _cache[key] = _Runner(_build_stage_a(merged=merged))
        _cache["a"] = _cache[key]
    if "b" not in _cache:
        _cache["b"] = _Runner(_build_stage_b())
    return _cache[key], _cache["b"]


def _pack_cols(v, width):
    # (E,)-like flat -> (128, ET*width) per-partition column blocks
    a = np.asarray(v, np.float32).reshape(ET, 128, width)
    return np.ascontiguousarray(a.transpose(1, 0, 2).reshape(128, ET * width))


def kernel(**inputs):
    # The 2048-wide merged scan applies one per-partition A column to both
    # e-tiles of a block; valid only when A is constant across channels
    # (true for the standard Mamba init A_log = log(1..N) tiled). Verify on
    # the actual inputs and fall back to the unmerged build otherwise.
    merged = True
    for nm in ("A_log", "A_b_log"):
        a = -np.exp(np.asarray(inputs[nm], np.float32))
        if not np.allclose(a, a[0:1, :], rtol=1e-6, atol=1e-7):
            merged = False
    run_a, run_b = _programs(merged)
    f16 = np.float16
    hs = np.asarray(inputs["hidden_states"], np.float32)

    w_inT = np.ascontiguousarray(np.asarray(inputs["in_proj_w"], np.float32).T).astype(f16)
    out_wT = np.ascontiguousarray(np.asarray(inputs["out_proj_w"], np.float32).T).astype(f16)
    # norm_w/b are per-D; in (D,L) layout D is the partition dim -> column k = rows 128k..128k+127
    nw = np.ascontiguousarray(np.asarray(inputs["norm_w"], np.float32).reshape(DT, 128).T)
    nb = np.ascontiguousarray(np.asarray(inputs["norm_b"], np.float32).reshape(DT, 128).T)

    per_dir = {}
    for d, sfx in ((0, ""), (1, "_b")):
        alog = np.asarray(inputs["A_log" if d == 0 else "A_b_log"], np.float32)
        per_dir[d] = dict(
            xp_wT=np.ascontiguousarray(np.asarray(inputs["x_proj_w" + sfx], np.float32).T).astype(f16),
            dtp_wT=np.ascontiguousarray(np.asarray(inputs["dt_proj_w" + sfx], np.float32).T).astype(f16),
            convw=_pack_cols(inputs["conv_w" + sfx], KC),
            convb=_pack_cols(inputs["conv_b" + sfx], 1),
            dtb=_pack_cols(inputs["dt_proj_b" + sfx], 1),
            avals=_pack_cols(-np.exp(alog), N),
            dvec=_pack_cols(inputs["D_fwd" if d == 0 else "D_bwd"], 1),
        )

    in_maps = []
    for c in range(8):
        b, d = c % 4, c // 4
        h = hs[b] if d == 0 else hs[b, ::-1]
        in_maps.append(dict(
            hsT=np.ascontiguousarray(h.T).astype(f16),
            w_inT=w_inT, out_wT=out_wT, nw=nw, nb=nb,
            **per_dir[d],
        ))
    _cache["last_in_maps_a"] = in_maps
    res_a = run_a(in_maps)

    LH = L // 2
    nfw = np.asarray(inputs["normf_w"], np.float32).reshape(1, D)
    in_maps_b = []
    for c in range(8):
        b, half = c % 4, c // 4
        rows = slice(half * LH, (half + 1) * LH)
        yfT = res_a[b]["y_part"].T            # (L, D) f16
        yrT = res_a[b + 4]["y_part"][:, ::-1].T
        in_maps_b.append(dict(
            yf=np.ascontiguousarray(yfT[rows]),
            yr=np.ascontiguousarray(yrT[rows]),
            res=np.ascontiguousarray(hs[b, rows]).astype(f16),
            nfw=nfw,
        ))
    _cache["last_in_maps_b"] = in_maps_b
    res_b = run_b(in_maps_b)

    out = np.empty((B, L, D), np.float32)
    for c in range(8):
        b, half = c % 4, c // 4
        out[b, half * LH:(half + 1) * LH] = res_b[c]["out"]
    return out
